# revision 1
# baseline (speedup 1.0000x reference)
"""Trainium2 Bass kernel for nn_EpipolarWarpOperator (B=8, C=320, H=W=64, S=3).

Sharding: pure data parallelism — one batch element per NeuronCore (8 cores).

Per-core pipeline (fp16 on-chip storage, fp32 PSUM accumulate):
  host: epipolar geometry -> bilinear corner indices/weights; samples sorted
        by y-group; S matrix [128, 20480] (4 nnz/col, bilinear*1/3, padded to
        128-aligned groups); slab row-gather indices; unsort gather indices.
  A: slab(g_b) = x^T rows [64g, 64g+128) fetched by indexed dma_gather from
     HBM; val[128 tok, 320 ch] = S_b.T @ slab  (PE matmul per 128-token block)
  B: unsort: SBUF-source transpose dma_gather of val rows by sorted position
     (per sample s) -> channel-major [128, 3, npix]; mean over s on DVE,
     written into a zero-padded 66x66 layout.
  C: 3x3 conv as 9 shifted matmuls over (mchunk, kchunk, tap), bias+ReLU on
     ACT, DMA out.
"""

import numpy as np

B, C, H, W = 8, 320, 64, 64
NUM_SAMPLES = 3
HW = H * W
NBLK = 160            # fixed token-block budget (>= 12288 + 64*127 padded)
NPAD = NBLK * 128
CPAD = 384            # channel pad so gather elem = 768B (mult of 256)
XROWS = 4224          # x^T rows incl. pad (max slab row 4159)
PW, PH = W + 2, H + 2
MB = [(0, 128), (128, 128), (256, 64)]   # channel chunking

import os as _os
A_CH = int(_os.environ.get("K_ACH", "16"))
B_CH = int(_os.environ.get("K_BCH", "512"))
CONV_PAIR = _os.environ.get("K_PAIR", "1") == "1"
CONV_ILV = _os.environ.get("K_ILV", "0") == "1"
SCRATCH = int(_os.environ.get("K_SCRATCH", "16384"))

assert HW % B_CH == 0

# ---------------------------------------------------------------- host prep

def _rodrigues_np(rv):
    theta = np.sqrt((rv * rv).sum())
    r = rv / max(theta, 1e-12)
    I = np.eye(3, dtype=np.float32)
    K = np.array([[0, -r[2], r[1]], [r[2], 0, -r[0]], [-r[1], r[0], 0]],
                 dtype=np.float32)
    R = np.cos(theta) * I + (1 - np.cos(theta)) * np.outer(r, r) + np.sin(theta) * K
    return I if theta < 1e-6 else R


def fundamental_np(Ks, Kt, ps, pt):
    Fs = []
    for b in range(Ks.shape[0]):
        Rs = _rodrigues_np(ps[b, :3].astype(np.float32))
        Rt = _rodrigues_np(pt[b, :3].astype(np.float32))
        ts_, tt_ = ps[b, 3:].astype(np.float32), pt[b, 3:].astype(np.float32)
        R_rel = Rs @ Rt.T
        t_rel = ts_ - R_rel @ tt_
        z = np.float32(0)
        skew = np.array([[z, -t_rel[2], t_rel[1]],
                         [t_rel[2], z, -t_rel[0]],
                         [-t_rel[1], t_rel[0], z]], dtype=np.float32)
        E = skew @ R_rel
        inv_Ks = np.linalg.inv(Ks[b].astype(np.float32))
        inv_Kt = np.linalg.inv(Kt[b].astype(np.float32))
        Fs.append(inv_Kt.T @ E @ inv_Ks)
    return np.stack(Fs).astype(np.float32)


def geometry(F):
    k = np.arange(HW)
    px = (k % W).astype(np.float32)
    py = (k // W).astype(np.float32)
    P = np.stack([px, py, np.ones_like(px)])
    lines = F.T.astype(np.float32) @ P
    a, b_, c = lines[0], lines[1], lines[2]
    W1, H1 = np.float32(W - 1), np.float32(H - 1)
    EPS = np.float32(1e-10)
    x1 = np.clip(-c / (a + EPS), 0.0, W1)
    x2 = np.clip(-(b_ * H1 + c) / (a + EPS), 0.0, W1)
    y1 = np.clip(-c / (b_ + EPS), 0.0, H1)
    y2 = np.clip(-(a * W1 + c) / (b_ + EPS), 0.0, H1)
    t = np.linspace(0.0, 1.0, NUM_SAMPLES, dtype=np.float32)
    sx = x1[:, None] * (1 - t) + x2[:, None] * t
    sy = y1[:, None] * (1 - t) + y2[:, None] * t
    x0 = np.floor(sx)
    y0 = np.floor(sy)
    wx = (sx - x0).astype(np.float32)
    wy = (sy - y0).astype(np.float32)
    x0i = np.clip(x0, 0, W - 1).astype(np.int32)
    y0i = np.clip(y0, 0, H - 1).astype(np.int32)
    return x0i, y0i, wx, wy


def build_sort(x0i, y0i, wx, wy):
    """x0i/y0i/wx/wy: [npix, S] for one pixel range. Returns S weights,
    per-block slab group, per-sample padded position, used block count."""
    flat_y = y0i.reshape(-1)
    order = np.argsort(flat_y, kind='stable')
    S = np.zeros((128, NPAD), dtype=np.float32)
    pos = np.zeros(flat_y.size, dtype=np.int32)
    blk_g = np.zeros(NBLK, dtype=np.int32)
    cur = 0
    x0f = x0i.reshape(-1)
    wxf = wx.reshape(-1)
    wyf = wy.reshape(-1)
    third = np.float32(1.0 / 3.0)
    for g in range(H):
        sel = order[flat_y[order] == g]
        n = sel.size
        if n == 0:
            continue
        cols = cur + np.arange(n)
        pos[sel] = cols
        x0s = x0f[sel]
        wxs = wxf[sel]
        wys = wyf[sel]
        x1s = np.minimum(x0s + 1, W - 1)
        np.add.at(S, (x0s, cols), (1 - wys) * (1 - wxs) * third)
        np.add.at(S, (x1s, cols), (1 - wys) * wxs * third)
        np.add.at(S, (64 + x0s, cols), wys * (1 - wxs) * third)
        np.add.at(S, (64 + x1s, cols), wys * wxs * third)
        nb_lo = cur // 128
        cur = ((cur + n + 127) // 128) * 128
        blk_g[nb_lo:cur // 128] = g
    assert cur <= NPAD, cur
    npix = x0i.shape[0]
    return (S.astype(np.float16), blk_g, pos.reshape(npix, NUM_SAMPLES),
            cur // 128)


def wrap16(idx, n):
    t = idx.astype(np.int16).reshape(n // 16, 16).T
    return np.tile(t, (8, 1)).copy()


def prep_batch(xb, F, nhalves):
    """Sort each pixel range independently so the device can overlap
    phase A of range h+1 with phase B of range h."""
    x0i, y0i, wx, wy = geometry(F)
    xt = np.zeros((XROWS, CPAD), dtype=np.float16)
    xt[:HW, :C] = xb.reshape(C, HW).T.astype(np.float16)
    hp = HW // nhalves
    parts = []
    for h in range(nhalves):
        sl = slice(h * hp, (h + 1) * hp)
        parts.append(build_sort(x0i[sl], y0i[sl], wx[sl], wy[sl]))
    return dict(xt=xt, parts=parts)


def assemble_batch(d, nbh, nhalves):
    """Pack per-half sort data into device arrays for block budget nbh."""
    hp = HW // nhalves
    S = np.zeros((128, nhalves * nbh * 128), dtype=np.float16)
    blk_g = np.zeros(nhalves * nbh, dtype=np.int32)
    gsecs = []
    for h, (S_h, bg_h, pos_h, used_h) in enumerate(d['parts']):
        assert used_h <= nbh
        S[:, h * nbh * 128: h * nbh * 128 + nbh * 128] = S_h[:, :nbh * 128]
        blk_g[h * nbh: h * nbh + nbh] = bg_h[:nbh]
        for s in range(NUM_SAMPLES):
            gsecs.append(pos_h[:, s])
    # pre-gather slabs on the host: block b needs x^T rows [64g_b, 64g_b+128)
    # laid out partition-major so phase A is plain contiguous HWDGE streaming
    # (no SWDGE ring traffic at all in phase A)
    rows = 64 * blk_g[:, None] + np.arange(128)[None, :]      # [nblk, 128]
    xts = d["xt"][rows]                                       # [nblk, 128, CPAD]
    xts = np.ascontiguousarray(
        xts.transpose(1, 0, 2).reshape(128, -1))              # [128, nblk*CPAD]
    return {
        "xts": xts,
        "s_mat": S,
        "gat_idx": wrap16(np.concatenate(gsecs), HW * NUM_SAMPLES),
    }


def prep_weights(conv_w, conv_b):
    Wl = np.zeros((128, 3 * 9 * C), dtype=np.float16)
    for kc, (koff, ksz) in enumerate(MB):
        for tap in range(9):
            dy, dx = tap // 3 - 1, tap % 3 - 1
            for moff, msz in MB:
                blk = conv_w[moff:moff + msz, koff:koff + ksz, dy + 1, dx + 1]
                Wl[0:ksz, kc * 9 * C + tap * C + moff: kc * 9 * C + tap * C
                   + moff + msz] = blk.T.astype(np.float16)
    # paired kc=2 weights: rows 0:64 = tap (dy=0,dx), rows 64:128 = (dy=-1,dx)
    Wl2 = np.zeros((128, 3 * C), dtype=np.float16)
    for dxi, dx in enumerate((-1, 0, 1)):
        for moff, msz in MB:
            top = conv_w[moff:moff + msz, 256:320, 1, dx + 1]      # dy=0
            bot = conv_w[moff:moff + msz, 256:320, 0, dx + 1]      # dy=-1
            Wl2[0:64, dxi * C + moff: dxi * C + moff + msz] = \
                top.T.astype(np.float16)
            Wl2[64:128, dxi * C + moff: dxi * C + moff + msz] = \
                bot.T.astype(np.float16)
    bias = np.zeros((128, 3), dtype=np.float32)
    for mc, (moff, msz) in enumerate(MB):
        bias[0:msz, mc] = conv_b[moff:moff + msz].astype(np.float32)
    return Wl, Wl2, bias


# ------------------------------------------------------------- bass program

_NC_CACHE = {}


def build_program(reps=1, nblk=NBLK, nhalves=1):
    assert nblk % (A_CH * nhalves) == 0 and nblk <= NBLK
    key = (reps, nblk, nhalves)
    if key in _NC_CACHE:
        return _NC_CACHE[key]
    import concourse.bacc as bacc
    import concourse.mybir as mybir
    from concourse.tile import TileContext

    fp16 = mybir.dt.float16
    f32 = mybir.dt.float32
    i16 = mybir.dt.int16

    nc = bacc.Bacc(target_bir_lowering=False,
                   dynamic_dma_scratch_size=SCRATCH)
    xts_d = nc.dram_tensor("xts", [128, nblk * CPAD], fp16,
                           kind="ExternalInput")
    S = nc.dram_tensor("s_mat", [128, nblk * 128], fp16,
                       kind="ExternalInput")
    gidx_d = nc.dram_tensor("gat_idx", [128, 3 * HW // 16], i16,
                            kind="ExternalInput")
    wl_d = nc.dram_tensor("wl", [128, 3 * 9 * C], fp16, kind="ExternalInput")
    wl2_d = nc.dram_tensor("wl2", [128, 3 * C], fp16, kind="ExternalInput")
    bias_d = nc.dram_tensor("bias", [128, 3], f32, kind="ExternalInput")
    out_d = nc.dram_tensor("out", [C, HW], f32, kind="ExternalOutput")

    with TileContext(nc) as tc:
        with tc.tile_pool(name="const", bufs=1) as constp:
            wl = constp.tile([128, 3 * 9 * C], fp16)
            nc.sync.dma_start(out=wl[:], in_=wl_d[:])
            wl2 = constp.tile([128, 3 * C], fp16)
            nc.sync.dma_start(out=wl2[:], in_=wl2_d[:])
            bias_t = constp.tile([128, 3], f32)
            nc.sync.dma_start(out=bias_t[:], in_=bias_d[:])
            gidx = constp.tile([128, 3 * HW // 16], i16)
            nc.sync.dma_start(out=gidx[:], in_=gidx_d[:])

            def body(_it):
                with tc.tile_pool(name="val", bufs=1) as valp:
                    val = valp.tile([128, nblk * CPAD], fp16)
                    # zero the channel-pad region of every rank stripe (on
                    # DVE, keeping the Pool sequencer free for gather
                    # descriptor generation)
                    val3 = val.rearrange("p (b c) -> p b c", c=CPAD)
                    nc.vector.memset(val3[:, :, C:CPAD], 0.0)

                    # ---- phase A: sampling matmuls ----
                    with tc.tile_pool(name="slab", bufs=2) as slabp, \
                         tc.tile_pool(name="smat", bufs=2) as smatp, \
                         tc.tile_pool(name="psA", bufs=2, space="PSUM") as psA:
                        DR = 4   # blocks per drain group (4 psum banks)
                        for chk in range(nblk // A_CH):
                            nidx = A_CH * 128
                            slab = slabp.tile([128, A_CH * CPAD], fp16)
                            nc.sync.dma_start(
                                out=slab[:],
                                in_=xts_d[:, chk * A_CH * CPAD:
                                          (chk + 1) * A_CH * CPAD])
                            smat = smatp.tile([128, A_CH * 128], fp16)
                            nc.sync.dma_start(
                                out=smat[:],
                                in_=S[:, chk * nidx:(chk + 1) * nidx])
                            for g4 in range(A_CH // DR):
                                ps = psA.tile([128, DR, 512], f32)
                                for b4 in range(DR):
                                    b = g4 * DR + b4
                                    nc.tensor.matmul(
                                        ps[:, b4, 0:C],
                                        smat[:, b * 128:(b + 1) * 128],
                                        slab[:, b * CPAD:b * CPAD + C],
                                        start=True, stop=True)
                                blk0 = chk * A_CH + g4 * DR
                                if g4 % 3 != 2:
                                    nc.vector.tensor_copy(
                                        val3[:, blk0:blk0 + DR, 0:C],
                                        ps[:, :, 0:C])
                                else:
                                    nc.scalar.copy(
                                        val3[:, blk0:blk0 + DR, 0:C],
                                        ps[:, :, 0:C])

                    # ---- phase B: unsort + mean -> padded layout ----
                    with tc.tile_pool(name="samp", bufs=1) as sampp:
                        sampled = sampp.tile([128, 3 * PH * PW], fp16)
                        smp4 = sampled.rearrange("p (k r c) -> p k r c",
                                                 k=3, r=PH)
                        # zero only the pad borders (interior is overwritten)
                        nc.vector.memset(smp4[:, :, 0:1, :], 0.0)
                        nc.vector.memset(smp4[:, :, PH - 1:PH, :], 0.0)
                        nc.vector.memset(smp4[:, :, :, 0:1], 0.0)
                        nc.vector.memset(smp4[:, :, :, PW - 1:PW], 0.0)
                        # duplicated kc=2 plane for paired (dy=0,dy=-1) taps:
                        # partitions 0:64 hold D at offset 0, 64:128 at +PW
                        smp2d = sampp.tile([128, PH * PW + PW], fp16)
                        with tc.tile_pool(name="gout",
                                          bufs=(2 if nblk <= 144 else 1)) \
                                as goutp:
                            rows_per = B_CH // W
                            nbh = nblk // nhalves
                            hp = HW // nhalves
                            nqh = hp // B_CH
                            for h in range(nhalves):
                                # unsort gathers read only this half's val
                                # slice, so they can start (and run under
                                # phase A of the next half) as soon as this
                                # half's sampling drains land.
                                valh = val[:, h * nbh * CPAD:
                                           (h + 1) * nbh * CPAD]
                                for q in range(nqh):
                                    gs = []
                                    for s in range(NUM_SAMPLES):
                                        c0 = ((h * 3 + s) * hp
                                              + q * B_CH) // 16
                                        g = goutp.tile([128, 3, B_CH], fp16,
                                                       name=f"g{s}",
                                                       tag=f"g{s}")
                                        nc.gpsimd.dma_gather(
                                            out_ap=g[:],
                                            in_ap=valh,
                                            idxs_ap=gidx[:, c0:
                                                         c0 + B_CH // 16],
                                            num_idxs=B_CH,
                                            num_idxs_reg=B_CH,
                                            elem_size=CPAD,
                                            transpose=True,
                                            sbuf_tokens_per_rank=128,
                                            sbuf_free_dim_per_rank=CPAD * 2,
                                        )
                                        gs.append(g)
                                    tmp = goutp.tile([128, 3 * B_CH], fp16,
                                                     tag="tmp")
                                    nc.vector.tensor_add(
                                        tmp[:],
                                        gs[0].rearrange("p k n -> p (k n)"),
                                        gs[1].rearrange("p k n -> p (k n)"))
                                    qg = h * nqh + q
                                    r0 = 1 + qg * rows_per
                                    nc.vector.tensor_add(
                                        smp4[:, :, r0:r0 + rows_per, 1:1 + W],
                                        tmp.rearrange("p (k r c) -> p k r c",
                                                      k=3, c=W),
                                        gs[2].rearrange("p k (r c) -> p k r c",
                                                        c=W))
                                    # band-wise pair-plane copy
                                    nq = HW // B_CH
                                    lo = r0 if qg > 0 else 0
                                    hi = (r0 + rows_per if qg < nq - 1
                                          else PH)
                                    band = smp4[0:64, 2, lo:hi, :].rearrange(
                                        "p a b -> p (a b)")
                                    nc.sync.dma_start(
                                        out=smp2d[0:64, lo * PW:hi * PW],
                                        in_=band)
                                    nc.sync.dma_start(
                                        out=smp2d[64:128,
                                                  (lo + 1) * PW:(hi + 1) * PW],
                                        in_=band)

                        nc.vector.memset(smp2d[64:128, 0:PW], 0.0)

                        # ---- phase C: 3x3 conv + bias + relu ----
                        # 24 matmuls per (m, r): 18 full-K taps (kc 0/1),
                        # 3 paired kc=2 (dy 0/-1 stacked), 3 single (dy=+1).
                        with tc.tile_pool(name="psC", bufs=2, space="PSUM") \
                                as psC, \
                             tc.tile_pool(name="outp", bufs=4) as outp:
                            NCOL = 512
                            rows_n = NCOL // W
                            NR = HW // NCOL
                            N_MM = 24 if CONV_PAIR else 27
                            smp2f = smp2d  # [128, PH*PW + PW]
                            smp2v = smp2f.rearrange("p (r c) -> p r c",
                                                    c=PW)
                            # collect the 24 (lhsT, rhs) generators per
                            # m-chunk, then issue r-outer so the PE consumes
                            # sampled bands as phase B streams them
                            mqs = []
                            for mc, (moff, msz) in enumerate(MB):
                                mmq = []
                                kcs = (0, 1) if CONV_PAIR else (0, 1, 2)
                                for tap in range(9):
                                    dy, dx = tap // 3 - 1, tap % 3 - 1
                                    for kc in kcs:
                                        ksz = 128 if kc < 2 else 64
                                        lhsT = wl[0:ksz,
                                                  kc * 9 * C + tap * C + moff:
                                                  kc * 9 * C + tap * C + moff + msz]
                                        mmq.append((lhsT,
                                            lambda r, dy=dy, dx=dx, kc=kc,
                                            ksz=ksz:
                                            smp4[0:ksz, kc,
                                                 1 + dy + r * rows_n:
                                                 1 + dy + r * rows_n + rows_n,
                                                 1 + dx:1 + dx + W]))
                                if CONV_PAIR:
                                    for dxi, dx in enumerate((-1, 0, 1)):
                                        # paired dy=0 (p<64) + dy=-1 (p>=64)
                                        lhsT = wl2[0:128,
                                                   dxi * C + moff:
                                                   dxi * C + moff + msz]
                                        mmq.append((lhsT, lambda r, dx=dx:
                                            smp2v[0:128,
                                                  1 + r * rows_n:
                                                  1 + r * rows_n + rows_n,
                                                  1 + dx:1 + dx + W]))
                                        # single dy=+1, K=64
                                        lhsT1 = wl[0:64,
                                                   2 * 9 * C + (2 * 3 + dxi) * C + moff:
                                                   2 * 9 * C + (2 * 3 + dxi) * C
                                                   + moff + msz]
                                        mmq.append((lhsT1, lambda r, dx=dx:
                                            smp4[0:64, 2,
                                                 2 + r * rows_n:
                                                 2 + r * rows_n + rows_n,
                                                 1 + dx:1 + dx + W]))
                                assert len(mmq) == N_MM
                                mqs.append((moff, msz, mmq))
                            for r in range(NR):
                                for mc, (moff, msz, mmq) in enumerate(mqs):
                                    ps = psC.tile([128, NCOL], f32,
                                                  name=f"ps{mc}",
                                                  tag=f"ps{mc}")
                                    for j, (lhsT, rhs_fn) in enumerate(mmq):
                                        nc.tensor.matmul(
                                            ps[0:msz], lhsT, rhs_fn(r),
                                            start=(j == 0),
                                            stop=(j == N_MM - 1))
                                    ot = outp.tile([128, NCOL], f32)
                                    nc.scalar.activation(
                                        ot[0:msz], ps[0:msz],
                                        mybir.ActivationFunctionType.Relu,
                                        bias=bias_t[0:msz, mc:mc + 1])
                                    nc.sync.dma_start(
                                        out=out_d[moff:moff + msz,
                                                  r * NCOL:(r + 1) * NCOL],
                                        in_=ot[0:msz])

            if reps == 1:
                body(0)
            else:
                with tc.For_i(0, reps, 1) as it:
                    body(it)

    nc.finalize()
    _NC_CACHE[key] = nc
    return nc


# ---------------------------------------------------------------- interface

def make_in_maps(x, source_intrinsics, target_intrinsics, source_pose,
                 target_pose, conv_w, conv_b):
    F = fundamental_np(source_intrinsics, target_intrinsics,
                       source_pose, target_pose)
    Wl, Wl2, bias = prep_weights(conv_w, conv_b)
    nhalves = 2
    ds = [prep_batch(x[b], F[b], nhalves) for b in range(B)]
    nbh = max(p[3] for d in ds for p in d['parts'])
    nbh = ((nbh + A_CH - 1) // A_CH) * A_CH
    if nhalves * nbh > NBLK:
        # half-split padding too large for the SBUF budget; fall back to
        # one global sort
        nhalves = 1
        ds = [prep_batch(x[b], F[b], nhalves) for b in range(B)]
        nbh = max(p[3] for d in ds for p in d['parts'])
        nbh = min(NBLK, ((nbh + A_CH - 1) // A_CH) * A_CH)
    in_maps = []
    for b in range(B):
        m = assemble_batch(ds[b], nbh, nhalves)
        m.update({"wl": Wl, "wl2": Wl2, "bias": bias})
        in_maps.append(m)
    return in_maps, nhalves * nbh, nhalves


def kernel(x, source_intrinsics, target_intrinsics, source_pose,
           target_pose, conv_w, conv_b, _reps=1):
    from concourse.bass_utils import run_bass_kernel_spmd
    x = np.asarray(x, dtype=np.float32)
    in_maps = make_in_maps(
        x, np.asarray(source_intrinsics), np.asarray(target_intrinsics),
        np.asarray(source_pose), np.asarray(target_pose),
        np.asarray(conv_w, dtype=np.float32), np.asarray(conv_b, dtype=np.float32))
    in_maps, nblk, nhalves = in_maps
    nc = build_program(_reps, nblk, nhalves)
    res = run_bass_kernel_spmd(nc, in_maps, list(range(8)))
    out = np.stack([res.results[i]["out"].reshape(C, H, W) for i in range(8)])
    return out.astype(np.float32)

